# revision 1
# baseline (speedup 1.0000x reference)
"""Trainium2 Bass kernel for AceStepAttention (B=2, S=2048, D=2048, H=16, KVH=4, HD=128).

Sharding: 8 cores = (batch 2) x (kv-head group 4). Each core computes, for its
batch b and kv group g: the 4 query heads [4g..4g+4) + kv head g, full
non-causal attention over S=2048, and the o_proj partial for its 512 columns
of o_w. Host sums the 4 partials per batch.

Per-core dataflow (v2 — staged to keep the PE continuously fed):
  Stage A (DMA-paced): K+V projection, contraction-chunk-major. All 16 token
    chunks accumulate simultaneously in PSUM (16 x [tok,256] accumulators =
    8 banks), consuming each hT chunk as it lands. PSUM evacuations are paired
    per bank so no evacuation blocks a later accumulating matmul (bank-granular
    WAR deps). The k RMS factors are batched into one Newton pass (rk [128,16])
    and applied as a per-partition scale on the scores Exp (rope and scores are
    linear in k, so the scale commutes out).
  Stage B (PE-paced): Q projection token-chunk-major in groups of 4 (hT now
    resident), norm+rope chains deferred one group, PE-transpose to QT. The
    LAST group's chains+transposes are deferred into stage C (they feed
    q-window 1 only) so they don't gate stage C's start.
  Stage C: per (head, q-window): scoresT = KT_chunk^T.QT -> Exp(scale=rk) ->
    probsT bf16; attnT += V^T.probsT; denominator via DVE quad-sums + one
    ones-matmul; o_proj of window i interleaved into window i+1's kc loop.
    Heads are software-pipelined: the next head's first score tiles are
    emitted before the current head's denominator matmul, hiding the DVE
    tree latency and the at-bank handoff (pat bufs=2).
Host: out[b] = sum_g outT(core b,g)^T.
"""

import numpy as np
import ml_dtypes

import concourse.bacc as bacc
import concourse.bass as bass
import concourse.mybir as mybir
from concourse import tile
from concourse.bass_utils import run_bass_kernel_spmd
from concourse.masks import make_identity

BF16 = mybir.dt.bfloat16
F32 = mybir.dt.float32
AF = mybir.ActivationFunctionType
ALU = mybir.AluOpType

B = 2
S = 2048
D = 2048
H = 16
KVH = 4
HD = 128
G = H // KVH          # q heads per core
N_CORES = 8
EPS = 1e-6
P = 128

INTERLEAVE_OPROJ = True
INC_DENOM = True
PB_BUFS = 4
HT_SPLIT = (1, 1, 1, 1, 2, 2, 4, 4)   # hT DMA chunk grouping (sum = dc_n)
QGRP = 4                        # stage-B token chunks per psum group
PREFETCH = 5                    # next-head sc units emitted before dn
DEFER_LAST = True               # defer last q group's chains into stage C
LA = 2                          # at-matmul lookahead behind scores/Exp


def build_program(nc, s=S, d=D, reps=1):
    """Emit the per-core SPMD program into nc (a Bacc). Returns nothing."""
    tc_n = s // P        # token chunks
    dc_n = d // P        # hidden-dim contraction chunks
    qw = min(1024, s)    # exp/psum q-window width
    nqp = s // qw
    nj = qw // 512       # 512-wide matmul sub-tiles per window
    assert tc_n % 4 == 0 and sum(HT_SPLIT) == dc_n

    hT = nc.dram_tensor("hT", [d, s], BF16, kind="ExternalInput")
    qwT = nc.dram_tensor("qwT", [d, G * HD], BF16, kind="ExternalInput")
    kwT = nc.dram_tensor("kwT", [d, HD], BF16, kind="ExternalInput")
    vwT = nc.dram_tensor("vwT", [d, HD], BF16, kind="ExternalInput")
    owT = nc.dram_tensor("owT", [G * HD, d], BF16, kind="ExternalInput")
    cwq = nc.dram_tensor("cwq", [P, s], BF16, kind="ExternalInput")
    swq = nc.dram_tensor("swq", [P, s], BF16, kind="ExternalInput")
    cwk = nc.dram_tensor("cwk", [P, s], BF16, kind="ExternalInput")
    swk = nc.dram_tensor("swk", [P, s], BF16, kind="ExternalInput")
    outT = nc.dram_tensor("outT", [d, s], F32, kind="ExternalOutput")

    import contextlib

    with tile.TileContext(nc) as tc:
        loop_ctx = tc.For_i(0, reps, 1) if reps > 1 else contextlib.nullcontext()
        with loop_ctx, tc.tile_pool(name="persist", bufs=1) as pp, \
                tc.tile_pool(name="work1", bufs=2) as wp:
            ident = pp.tile([P, P], BF16, tag="ident")
            make_identity(nc, ident[:, :])
            ones = pp.tile([P, P], BF16, tag="ones")
            nc.vector.memset(ones[:, :], 1.0)
            # touch the ACT function tables during the startup DMA stall so
            # the first real Square/Sqrt/Exp pays no table-load
            warm = pp.tile([P, 2], F32, tag="warm")
            nc.vector.memset(warm[:, :], 1.0)
            for fn_ in (AF.Square, AF.Sqrt, AF.Exp):
                nc.scalar.activation(warm[:, 0:1], warm[:, 1:2], fn_)

            owT1 = pp.tile([P, G, d], BF16, tag="owT1", name="owT1")
            QTall = pp.tile([P, G + 1, s], BF16, tag="QTall", name="QTall")
            KVE = pp.tile([P, tc_n, 2 * HD], BF16, tag="KVE")   # [k | v] per chunk
            attnT = [pp.tile([P, s], BF16, tag=f"attnT{h}", name=f"attnT{h}") for h in range(G)]
            rk = pp.tile([P, tc_n], F32, tag="rk")   # k rsqrt factors (Exp scale)

            # tables live in wp (outlives p1) — the last q chunks' rope chains
            # run inside stage C, after p1 closes.
            tabs = {}
            for name in ("cwk", "swk", "cwq", "swq"):
                tabs[name] = wp.tile([P, tc_n, HD], BF16, tag=name, name=name, bufs=1)

            deferred_q = []   # chain closures for the last stage-B group
            deferred_tr = []  # transpose closures deferred into stage C
            # ---------------- Stage A+B: projections -------------------------
            with tc.tile_pool(name="p1data", bufs=1) as p1:
                # --- input DMAs, interleaved so stage A's consumption is fed
                # first: kvw0, hT0, kvw1k, hT1, kvw1v, hT2.., then qwT/tables.
                # All on the sync queue: issue order == service order. owT's
                # DMA is issued from the sync queue at stage C emission so its
                # 2MB never delays the hT stream.
                kvw0 = p1.tile([P, 2, 2 * HD], BF16, tag="kvw0", name="kvw0")
                kvw1 = p1.tile([P, dc_n - 2, 2 * HD], BF16, tag="kvw1", name="kvw1")
                hTt = []
                hTd = []
                c0 = 0
                for i, sz in enumerate(HT_SPLIT):
                    t_ = p1.tile([P, sz, s], BF16, tag=f"hT_{i}", name=f"hT_{i}")
                    hTt.append((c0, t_))
                    hTd.append((t_, hT.ap().rearrange("(i p) n -> p i n", p=P)[:, c0:c0 + sz, :]))
                    c0 += sz
                nc.sync.dma_start(kvw0[:, :, 0:HD],
                                  kwT.ap().rearrange("(i p) n -> p i n", p=P)[:, 0:2, :])
                nc.sync.dma_start(kvw0[:, :, HD:2 * HD],
                                  vwT.ap().rearrange("(i p) n -> p i n", p=P)[:, 0:2, :])
                nc.sync.dma_start(hTd[0][0][:, :, :], hTd[0][1])
                nc.sync.dma_start(kvw1[:, :, 0:HD],
                                  kwT.ap().rearrange("(i p) n -> p i n", p=P)[:, 2:dc_n, :])
                nc.sync.dma_start(hTd[1][0][:, :, :], hTd[1][1])
                nc.sync.dma_start(kvw1[:, :, HD:2 * HD],
                                  vwT.ap().rearrange("(i p) n -> p i n", p=P)[:, 2:dc_n, :])
                for t_, ap_ in hTd[2:]:
                    nc.sync.dma_start(t_[:, :, :], ap_)

                def kvw_t(c):
                    return kvw0[:, c, :] if c < 2 else kvw1[:, c - 2, :]

                def hT_t(c):
                    for c0, t_ in hTt:
                        if c0 <= c < c0 + t_.shape[1]:
                            return t_[:, c - c0, :]
                    raise IndexError(c)

                assert dc_n % 8 == 0
                qwT8 = [p1.tile([P, 8, G * HD], BF16, tag=f"qwT8_{i}", name=f"qwT8_{i}")
                        for i in range(2)]
                nc.sync.dma_start(qwT8[0][:, :, :],
                                  qwT.ap().rearrange("(i p) n -> p i n", p=P)[:, 0:8, :])
                for name, dram in (("cwk", cwk), ("swk", swk)):
                    nc.sync.dma_start(tabs[name][:, :, :],
                                      dram.ap().rearrange("p (c d) -> p c d", d=HD))
                nc.sync.dma_start(qwT8[1][:, :, :],
                                  qwT.ap().rearrange("(i p) n -> p i n", p=P)[:, 8:16, :])
                for name, dram in (("cwq", cwq), ("swq", swq)):
                    nc.sync.dma_start(tabs[name][:, :, :],
                                      dram.ap().rearrange("p (c d) -> p c d", d=HD))

                def qwT_t(c):
                    return qwT8[c // 8][:, c % 8, :]

                # ---- Stage A: K+V projection, contraction-chunk-major -------
                ssqk = wp.tile([P, tc_n], F32, tag="ssqk", bufs=1)
                pvk = [KVE[:, t, 0:HD] for t in range(tc_n)]
                with tc.tile_pool(name="psum_kv", bufs=1, space="PSUM") as pkv:
                    # two token chunks share each PSUM bank, so a matmul
                    # start=True of one region would wipe the other (start
                    # zeroes bank-granular). Instead pre-zero the banks on DVE
                    # and accumulate with start=False throughout.
                    kvp = [pkv.tile([P, 2, 2 * HD], F32, tag=f"kv{i}", name=f"kv{i}")
                           for i in range(tc_n // 2)]
                    for i in range(tc_n // 2):
                        nc.vector.memset(kvp[i][:, :, :], 0.0)

                    def kv_ps(t):
                        return kvp[t // 2][:, t % 2, :]

                    def evac_kv(t):
                        # one wide copy (k|v together) per chunk, alternating
                        # engines: the kv banks release for stage B asap
                        if t % 2 == 0:
                            nc.scalar.copy(KVE[:, t, :], kv_ps(t)[:, :])
                        else:
                            nc.vector.tensor_copy(KVE[:, t, :], kv_ps(t)[:, :])

                    for c in range(dc_n):
                        st = dict(start=False, stop=(c == dc_n - 1),
                                  skip_group_check=True)
                        for t in range(tc_n):
                            ts_ = slice(t * P, (t + 1) * P)
                            nc.tensor.matmul(kv_ps(t), lhsT=hT_t(c)[:, ts_],
                                             rhs=kvw_t(c), **st)
                            # evacuate per PSUM-bank pair as soon as both
                            # chunks of the bank close (bank-granular WAR:
                            # an earlier evac would stall the odd chunk)
                            if c == dc_n - 1 and t % 2 == 1:
                                evac_kv(t - 1)
                                evac_kv(t)

                for t in range(tc_n):
                    sqsk = wp.tile([P, HD], F32, tag="sqsk", name=f"sqsk{t}")
                    nc.scalar.activation(sqsk[:, :], pvk[t][:, :], AF.Square,
                                         scale=float(HD ** -0.5),
                                         accum_out=ssqk[:, t:t + 1])
                # k rsqrt factors: one batched Newton pass over all chunks
                nc.vector.tensor_scalar_add(ssqk[:, :], ssqk[:, :], float(EPS))
                t1k = wp.tile([P, tc_n], F32, tag="t1k", bufs=1)
                nc.scalar.activation(t1k[:, :], ssqk[:, :], AF.Sqrt)
                nc.vector.reciprocal(rk[:, :], t1k[:, :])
                nc.vector.tensor_tensor(t1k[:, :], rk[:, :], rk[:, :], op=ALU.mult)
                nc.vector.tensor_tensor(t1k[:, :], t1k[:, :], ssqk[:, :], op=ALU.mult)
                nc.vector.tensor_scalar(t1k[:, :], t1k[:, :], -0.5, 1.5,
                                        op0=ALU.mult, op1=ALU.add)
                nc.vector.tensor_tensor(rk[:, :], rk[:, :], t1k[:, :], op=ALU.mult)

                # k rope chains, all-DVE (the rotate-half sign is folded into
                # the host swk table's first half; the halves are read with
                # offset APs instead of an ACT pre-rotate)
                kfs = []
                HH = HD // 2
                for t in range(tc_n):
                    ta = wp.tile([P, HD], BF16, tag="kta", name=f"kta{t}", bufs=3)
                    nc.vector.tensor_tensor(ta[:, :], pvk[t][:, :], tabs["cwk"][:, t, :], op=ALU.mult)
                    kr = wp.tile([P, HD], BF16, tag="krot", name=f"krot{t}", bufs=3)
                    nc.vector.tensor_tensor(kr[:, 0:HH], pvk[t][:, HH:HD],
                                            tabs["swk"][:, t, 0:HH], op=ALU.mult)
                    nc.vector.tensor_tensor(kr[:, HH:HD], pvk[t][:, 0:HH],
                                            tabs["swk"][:, t, HH:HD], op=ALU.mult)
                    kf = wp.tile([P, HD], BF16, tag=f"kf{t}", name=f"kf{t}", bufs=1)
                    nc.vector.tensor_tensor(kf[:, :], ta[:, :], kr[:, :], op=ALU.add)
                    kfs.append(kf)

                # ---- Stage B: Q projection (hT resident), groups of QGRP ----
                with (
                    tc.tile_pool(name="psum_q", bufs=1, space="PSUM") as pq,
                    tc.tile_pool(name="psum_tr", bufs=2, space="PSUM") as ptr,
                ):
                    def k_transpose(t):
                        pstk = ptr.tile([P, HD], BF16, tag="pstk", name=f"pstk{t}")
                        nc.tensor.transpose(pstk[:, :], kfs[t][:, :], ident[:, :])
                        nc.scalar.copy(QTall[:, G, t * P:(t + 1) * P], pstk[:, :])

                    def q_newton(gi, ssq, n):
                        # batched rsqrt for a whole group (n = chunks*heads)
                        nc.vector.tensor_scalar_add(ssq[:, 0:n], ssq[:, 0:n], float(EPS))
                        r0 = wp.tile([P, 16], F32, tag=f"r0g{gi}", name=f"r0g{gi}", bufs=1)
                        t1 = wp.tile([P, 16], F32, tag="t1g", name=f"t1g{gi}", bufs=2)
                        nc.scalar.activation(t1[:, 0:n], ssq[:, 0:n], AF.Sqrt)
                        nc.vector.reciprocal(r0[:, 0:n], t1[:, 0:n])
                        nc.vector.tensor_tensor(t1[:, 0:n], r0[:, 0:n], r0[:, 0:n], op=ALU.mult)
                        nc.vector.tensor_tensor(t1[:, 0:n], t1[:, 0:n], ssq[:, 0:n], op=ALU.mult)
                        nc.vector.tensor_scalar(t1[:, 0:n], t1[:, 0:n], -0.5, 1.5,
                                                op0=ALU.mult, op1=ALU.add)
                        nc.vector.tensor_tensor(r0[:, 0:n], r0[:, 0:n], t1[:, 0:n], op=ALU.mult)
                        nc.vector.tensor_scalar_mul(r0[:, 0:n], r0[:, 0:n], float(HD ** -0.5))
                        return r0

                    def q_chain(t, src, r0, i):
                        # scale by rsqrt + all-DVE rope (sign-folded swq),
                        # src is the psum tile (stage B) or a pv copy (stage C)
                        qs = wp.tile([P, G * HD], BF16, tag="qs", name=f"qs{t}", bufs=2)
                        qs3 = qs[:, :].rearrange("p (h x) -> p h x", h=G)
                        src3 = src.rearrange("p (h x) -> p h x", h=G)
                        r0b = r0[:, i * G:(i + 1) * G, None].to_broadcast([P, G, HD])
                        nc.vector.tensor_tensor(qs3[:, :, :], src3[:, :, :], r0b, op=ALU.mult)
                        ta = wp.tile([P, G * HD], BF16, tag="qta", name=f"qta{t}", bufs=2)
                        ta3 = ta[:, :].rearrange("p (h x) -> p h x", h=G)
                        rot = wp.tile([P, G * HD], BF16, tag="rot", name=f"rot{t}", bufs=2)
                        r3 = rot[:, :].rearrange("p (h x) -> p h x", h=G)
                        sw_b = tabs["swq"][:, t:t + 1, :].to_broadcast([P, G, HD])
                        cw_b = tabs["cwq"][:, t:t + 1, :].to_broadcast([P, G, HD])
                        nc.vector.tensor_tensor(ta3[:, :, :], qs3[:, :, :], cw_b, op=ALU.mult)
                        nc.vector.tensor_tensor(r3[:, :, 0:HH], qs3[:, :, HH:HD],
                                                sw_b[:, :, 0:HH], op=ALU.mult)
                        nc.vector.tensor_tensor(r3[:, :, HH:HD], qs3[:, :, 0:HH],
                                                sw_b[:, :, HH:HD], op=ALU.mult)
                        qf = wp.tile([P, G * HD], BF16, tag=f"qf{t}", name=f"qf{t}", bufs=1)
                        qf3 = qf[:, :].rearrange("p (h x) -> p h x", h=G)
                        nc.vector.tensor_tensor(qf3[:, :, :], ta3[:, :, :], r3[:, :, :], op=ALU.add)

                        def do_transpose(mk_pst, t=t, qf=qf):
                            ts_ = slice(t * P, (t + 1) * P)
                            pst = mk_pst(t)
                            for h in range(G):
                                hs = slice(h * HD, (h + 1) * HD)
                                nc.tensor.transpose(pst[:, hs], qf[:, hs], ident[:, :])
                            pst3 = pst[:, 0:G * HD].rearrange("p (h x) -> p h x", h=G)
                            nc.scalar.copy(QTall[:, 0:G, ts_], pst3[:, :, :])
                        return do_transpose

                    def finish_q_group(gi, grp, pss, defer=False):
                        """Norm+rope for a group. Non-deferred: squares and the
                        scale multiply read PSUM directly (no pv copy), all
                        emitted now; returns transpose closures. Deferred (last
                        group): evacuate pv now, return closures whose stage-C
                        part is DVE-only except the squares."""
                        ssq = wp.tile([P, 16], F32, tag=f"ssqg{gi}", name=f"ssqg{gi}", bufs=1)
                        sqs = wp.tile([P, HD], F32, tag="sqs", name=f"sqs{grp[0]}", bufs=2)

                        def squares(srcs, i0=0):
                            for i in range(len(srcs)):
                                for h in range(G):
                                    hs = slice(h * HD, (h + 1) * HD)
                                    nc.scalar.activation(sqs[:, :], srcs[i][:, hs], AF.Square,
                                                         scale=float(HD ** -0.5),
                                                         accum_out=ssq[:, (i0 + i) * G + h:(i0 + i) * G + h + 1])

                        # evacuate PSUM immediately (fast slot release for the
                        # next group / stage C); everything else reads pv
                        pvs = []
                        for i, t in enumerate(grp):
                            pv = wp.tile([P, G * HD], F32, tag="pv", name=f"pv{t}", bufs=6)
                            if t % 2 == 0:
                                nc.scalar.copy(pv[:, :], pss[i][:, :])
                            else:
                                nc.vector.tensor_copy(pv[:, :], pss[i][:, :])
                            pvs.append(pv)
                        if not defer:
                            srcs = [pv[:, :] for pv in pvs]
                            squares(srcs)
                            r0 = q_newton(gi, ssq, len(grp) * G)
                            # emits the chains now; returns transpose closures
                            return [q_chain(t, srcs[i], r0, i) for i, t in enumerate(grp)]
                        # squares+newton emit now (keeps ACT-table thrash out
                        # of stage C's Exp stream); the DVE-only scale/rope
                        # chains defer into stage C
                        squares([pv[:, :] for pv in pvs])
                        r0 = q_newton(gi, ssq, len(grp) * G)
                        return [lambda pv=pv, t=t, i=i: q_chain(t, pv[:, :], r0, i)
                                for i, (t, pv) in enumerate(zip(grp, pvs))]

                    def mk_pst_b(t):
                        return ptr.tile([P, G * HD], BF16, tag="pst", name=f"pst{t}")[:, :]

                    pending_tr = []
                    ngrp = (tc_n + QGRP - 1) // QGRP
                    for gi in range(ngrp):
                        grp = list(range(gi * QGRP, min((gi + 1) * QGRP, tc_n)))
                        pss = [pq.tile([P, G * HD], F32, tag=f"ps{i}",
                                       name=f"ps{i}_{grp[0]}") for i in range(len(grp))]
                        for c in range(dc_n):
                            st = dict(start=(c == 0), stop=(c == dc_n - 1))
                            for i, t in enumerate(grp):
                                ts_ = slice(t * P, (t + 1) * P)
                                nc.tensor.matmul(pss[i][:, :], lhsT=hT_t(c)[:, ts_],
                                                 rhs=qwT_t(c), **st)
                        # k transposes ride between groups (rope done by now)
                        if gi == 0:
                            for t in range(tc_n // 2):
                                k_transpose(t)
                        elif gi == 1:
                            for t in range(tc_n // 2, tc_n):
                                k_transpose(t)
                        if gi < ngrp - 1 or not DEFER_LAST:
                            for fn in pending_tr:
                                fn(mk_pst_b)
                            pending_tr = []
                        if gi == ngrp - 1 and DEFER_LAST:
                            # defer norm+rope into stage C: emitting it here
                            # would gate stage C's start on this group's
                            # ACT/DVE backlog
                            deferred_q.extend(finish_q_group(gi, grp, pss, defer=True))
                        else:
                            pending_tr = finish_q_group(gi, grp, pss)
                    if DEFER_LAST:
                        # the second-to-last group's transposes (chains already
                        # emitted) and the last group's whole chains are
                        # deferred into stage C — they feed q-window 1 only
                        deferred_tr.extend(pending_tr)
                    else:
                        for fn in pending_tr:
                            fn(mk_pst_b)

            # ---------------- Stage C (+interleaved o_proj) ------------------
            with (
                # pat first: its banks overlap stage B's pq (released early);
                # psc lands on ptr's banks + the two banks B never used
                tc.tile_pool(name="psum_at", bufs=2, space="PSUM") as pat,
                tc.tile_pool(name="psum_sc", bufs=2, space="PSUM") as psc,
                tc.tile_pool(name="work2", bufs=3) as wp2,
                tc.tile_pool(name="work3", bufs=4) as wp3,
            ):
                # issued from the sync queue: services after all input DMAs
                nc.sync.dma_start(owT1[:, :, :], owT.ap().rearrange("(g p) n -> p g n", p=P))

                def mk_pst_c(t):
                    return psc.tile([P, 2 * qw], BF16, tag="sc",
                                    name=f"pstc{t}")[:, 0:G * HD]

                def oproj_tile(oc, qp, drain=False):
                    os_ = slice(oc * P, (oc + 1) * P)
                    ob = wp3.tile([P, qw], F32, tag="ob", name=f"ob{oc}_{qp}", bufs=3)
                    for j in range(nj):
                        qc = qp * nj + j
                        qs_ = slice(qc * 512, (qc + 1) * 512)
                        # in the drain there are no more heads: rotate through
                        # the at pool too for 4 effective slots (the evac copy
                        # never blocks the next tile's matmuls)
                        use_pat = (oc % 2 == 1) if drain else (oc % 4 == 1)
                        if use_pat:
                            ot = pat.tile([P, qw], F32, tag="at", name=f"ot{oc}_{qc}")
                        else:
                            ot = psc.tile([P, qw], F32, tag="sc", name=f"ot{oc}_{qc}")
                        for g in range(G):
                            nc.tensor.matmul(ot[:, 0:512], lhsT=owT1[:, g, os_],
                                             rhs=attnT[g][:, qs_],
                                             start=(g == 0), stop=(g == G - 1))
                        if oc % 2 == 0:
                            nc.vector.tensor_copy(ob[:, j * 512:(j + 1) * 512], ot[:, 0:512])
                        else:
                            nc.scalar.copy(ob[:, j * 512:(j + 1) * 512], ot[:, 0:512])
                    nc.gpsimd.dma_start(outT.ap()[os_, qp * qw:(qp + 1) * qw], ob[:, :])

                pending = []          # deferred oproj tiles
                pending_ctr = deferred_tr  # deferred q transposes (stage-B tail)

                def attn_head(h, qp):
                    at = pat.tile([P, qw], F32, tag="at", name=f"at{h}_{qp}")
                    pb0 = pbq = pbsum = pbf = None
                    quads = []
                    pbs = {}
                    for kci in range(tc_n + LA):
                        if kci < tc_n:
                            ks_ = slice(kci * P, (kci + 1) * P)
                            sc = psc.tile([P, qw], F32, tag="sc", name=f"sc{h}_{qp}_{kci}")
                            for j in range(nj):
                                qs_ = slice(qp * qw + j * 512, qp * qw + (j + 1) * 512)
                                nc.tensor.matmul(sc[:, j * 512:(j + 1) * 512],
                                                 lhsT=QTall[:, G, ks_], rhs=QTall[:, h, qs_],
                                                 start=True, stop=True)
                            pbn = wp2.tile([P, qw], BF16, tag="pb", name=f"pb{h}_{qp}_{kci}", bufs=PB_BUFS)
                            nc.scalar.activation(pbn[:, :], sc[:, :], AF.Exp,
                                                 scale=rk[:, kci:kci + 1])
                            pbs[kci] = pbn
                        if kci < LA:
                            continue
                        kc = kci - LA
                        pb = pbs.pop(kc)
                        st = dict(start=(kc == 0), stop=(kc == tc_n - 1))
                        for j in range(nj):
                            js = slice(j * 512, (j + 1) * 512)
                            nc.tensor.matmul(at[:, js], lhsT=KVE[:, kc, HD:2 * HD], rhs=pb[:, js], **st)
                        # denominator: running quad-sums of probs on DVE (bf16)
                        iq = kc % 4
                        if iq == 0:
                            pb0 = pb
                        elif iq == 1:
                            pbq = wp2.tile([P, qw], BF16, tag=f"pbq{kc // 4}",
                                           name=f"pbq{h}_{qp}_{kc}", bufs=1)
                            nc.vector.tensor_tensor(pbq[:, :], pb0[:, :], pb[:, :], op=ALU.add)
                        else:
                            nc.vector.tensor_tensor(pbq[:, :], pbq[:, :], pb[:, :], op=ALU.add)
                        if iq == 3:
                            if not INC_DENOM or tc_n == 4:
                                quads.append(pbq)
                            elif kc == 3:
                                pbsum = wp2.tile([P, qw], F32, tag="pbsum",
                                                 name=f"pbsum{h}_{qp}", bufs=2)
                                nc.vector.tensor_copy(pbsum[:, :], pbq[:, :])
                            elif kc == tc_n - 1:
                                pbf = wp2.tile([P, qw], BF16, tag="pbf",
                                               name=f"pbf{h}_{qp}", bufs=2)
                                nc.vector.tensor_tensor(pbf[:, :], pbsum[:, :], pbq[:, :], op=ALU.add)
                            else:
                                nc.vector.tensor_tensor(pbsum[:, :], pbsum[:, :], pbq[:, :], op=ALU.add)
                        # interleave o_proj of the previous q-window and the
                        # stage-B tail's q chains/transposes
                        if INTERLEAVE_OPROJ and pending and kc % 4 == 2:
                            oproj_tile(*pending.pop(0))
                        if deferred_q and h >= 1 and kc % 4 == 1:
                            tr = deferred_q.pop(0)()
                            if tr is not None:
                                pending_ctr.append(tr)
                        elif pending_ctr and kc % 4 == 3:
                            pending_ctr.pop(0)(mk_pst_c)
                        yield "kc"
                    while len(quads) > 1:
                        nxt = []
                        for i in range(0, len(quads) - 1, 2):
                            dt_ = BF16 if len(quads) == 2 else F32
                            tsum = wp2.tile([P, qw], dt_, tag=f"pbt{len(quads)}_{i}",
                                            name=f"pbt{h}_{qp}_{len(quads)}_{i}", bufs=1)
                            nc.vector.tensor_tensor(tsum[:, :], quads[i][:, :],
                                                    quads[i + 1][:, :], op=ALU.add)
                            nxt.append(tsum)
                        if len(quads) % 2:
                            nxt.append(quads[-1])
                        quads = nxt
                    yield "pre_dn"
                    dnrhs = quads[0] if quads else pbf
                    dn = psc.tile([P, qw], F32, tag="sc", name=f"dn{h}_{qp}")
                    for j in range(nj):
                        js = slice(j * 512, (j + 1) * 512)
                        nc.tensor.matmul(dn[:, js], lhsT=ones[:, :], rhs=dnrhs[:, js],
                                         start=True, stop=True)
                    inv = wp2.tile([P, qw], F32, tag="inv", name=f"inv{h}_{qp}", bufs=2)
                    for j in range(nj):
                        js = slice(j * 512, (j + 1) * 512)
                        ws = slice(qp * qw + j * 512, qp * qw + (j + 1) * 512)
                        nc.vector.reciprocal_approx_fast(inv[:, js], dn[:, js])
                        nc.vector.tensor_tensor(attnT[h][:, ws], at[:, js], inv[:, js], op=ALU.mult)

                def run_to_predn(g):
                    for v in g:
                        if v == "pre_dn":
                            return
                    raise AssertionError("generator ended early")

                def exhaust(g):
                    for _ in g:
                        pass

                heads = [(h, qp) for qp in range(nqp) for h in range(G)]
                prev = None
                prev_hqp = None
                for h, qp in heads:
                    g = attn_head(h, qp)
                    if prev is not None:
                        # prefetch a few score units of this head before the
                        # previous head's denominator (hides the DVE tree
                        # latency and the at-bank handoff)
                        for _ in range(PREFETCH):
                            next(g)
                        exhaust(prev)
                        if prev_hqp[0] == G - 1:
                            # window prev_qp's attnT writes are all emitted
                            # now; its o_proj tiles may interleave from here
                            for oc in range(dc_n):
                                pending.append((oc, prev_hqp[1]))
                    run_to_predn(g)
                    prev, prev_hqp = g, (h, qp)
                exhaust(prev)
                for oc in range(dc_n):
                    pending.append((oc, prev_hqp[1]))
                for oc_qc in pending:
                    oproj_tile(*oc_qc, drain=True)


_COMPILED = {}


def _get_compiled(num_devices=N_CORES):
    key = num_devices
    if key not in _COMPILED:
        nc = bacc.Bacc("TRN2", target_bir_lowering=False, debug=False,
                       num_devices=num_devices)
        build_program(nc)
        nc.compile()
        _COMPILED[key] = nc
    return _COMPILED[key]


def _bf16(x):
    return np.ascontiguousarray(x).astype(ml_dtypes.bfloat16)


def prep_in_maps(hidden_states, cos, sin, q_w, k_w, v_w, o_w, q_norm_w, k_norm_w):
    """Shard + pre-transpose + cast the full inputs into 8 per-core maps."""
    hidden_states = np.asarray(hidden_states, np.float32)
    cos = np.asarray(cos, np.float32)
    sin = np.asarray(sin, np.float32)
    q_w = np.asarray(q_w, np.float32)
    k_w = np.asarray(k_w, np.float32)
    v_w = np.asarray(v_w, np.float32)
    o_w = np.asarray(o_w, np.float32)
    q_norm_w = np.asarray(q_norm_w, np.float32)
    k_norm_w = np.asarray(k_norm_w, np.float32)

    # norm weights folded into the rope tables (see module docstring)
    qn_rot = np.concatenate([q_norm_w[HD // 2:], q_norm_w[:HD // 2]])
    kn_rot = np.concatenate([k_norm_w[HD // 2:], k_norm_w[:HD // 2]])

    def _tab(x):
        # [S, HD] -> [128, S] chunk-major: row p holds [chunk0, chunk1, ...]
        return _bf16(x.reshape(S // P, P, HD).transpose(1, 0, 2).reshape(P, S))

    # rotate-half's minus sign is folded into the sin tables' first half:
    # rope(x) = x*cw + [x[h:] ; x[:h]]*sw_folded  (pure gather, no negate op)
    sgn = np.ones((HD,), np.float32)
    sgn[:HD // 2] = -1.0
    cwq = _tab(cos * q_norm_w[None, :])
    swq = _tab(sin * qn_rot[None, :] * sgn[None, :])
    cwk = _tab(cos * k_norm_w[None, :])
    swk = _tab(sin * kn_rot[None, :] * sgn[None, :])

    in_maps = []
    for c in range(N_CORES):
        b, g = c // KVH, c % KVH
        in_maps.append({
            "hT": _bf16(hidden_states[b].T),
            "qwT": _bf16(q_w[g * G * HD:(g + 1) * G * HD, :].T),
            "kwT": _bf16(k_w[g * HD:(g + 1) * HD, :].T),
            "vwT": _bf16(v_w[g * HD:(g + 1) * HD, :].T),
            "owT": _bf16(o_w[:, g * G * HD:(g + 1) * G * HD].T),
            "cwq": cwq, "swq": swq, "cwk": cwk, "swk": swk,
        })
    return in_maps


def kernel(**inputs):
    nc = _get_compiled()
    in_maps = prep_in_maps(**inputs)
    res = run_bass_kernel_spmd(nc, in_maps, core_ids=list(range(N_CORES)))
    out = np.zeros((B, S, D), np.float32)
    for c in range(N_CORES):
        out[c // KVH] += res.results[c]["outT"].T
    return out

